# revision 1
# baseline (speedup 1.0000x reference)
"""Additive (Bahdanau) attention as a TRN2 Bass/Tile kernel, SPMD over 8 cores.

Math per batch b (shapes: Q (256,256), K (1024,256), V (1024,256), H=128):
    qp = Q @ Wq.T                       (NQ, H)
    kp = K @ Wk.T                       (NKV, H)
    s[i, j]  = sum_h Wv[h] * tanh(qp[i, h] + kp[j, h])
    attn     = masked softmax_j(s)      (j < valid_lens[b])
    out      = attn @ V                 (NQ, DV)

Device decomposition: work is split into "slots" of 128 contiguous keys of one
batch.  Each slot computes the *unnormalized* softmax partials over its keys
for all 256 queries:

    num[i, :] = sum_j exp(s[i, j]) * V[j, :]      den[i] = sum_j exp(s[i, j])

The host sums the partials per batch and divides.  exp is applied without
max-subtraction (|s| <= sum_h |Wv[h]|, a few units, so exp cannot overflow),
which makes the partial sums mathematically exact under any key split.  That
lets the host:
  * skip key blocks that are entirely masked (j >= valid_lens[b]),
  * load-balance the surviving slots evenly across the 8 cores.
Masked keys inside a boundary slot contribute nothing because the host zeroes
their rows of [V | 1] (both the numerator rows and the ones column).

Per-slot device pipeline (H=128 lives on the SBUF partition axis):
  PE    kpT(128h,128j) / qpT(128h,256i) projections from host-pre-transposed
        K/Q chunks (layout prep only; the FLOPs happen on device)
  DVE   sums[h, jj, i] = qpT[h, i] + kpT[h, j]  in bf16
        (one tensor_scalar_add per key; ~277 ns/op is the kernel's wall)
  ACT   tanh over a whole 32-key group in one instruction (128 x 8192 bf16)
  PE    per key: stationary (Wv o+ I32) column, bf16 -> accumulates score
        rows into a 32-aligned (32, 256) fp32 PSUM block (col-tiling)
  ACT   exp over the slot's scoresT (128j, 256i), PSUM -> SBUF fp32
  PE    expT.T @ [V | 1] fp32 -> (128i, VE_W) partials, 2 query chunks
  DMA   PSUM -> SBUF (DVE copy) -> DRAM

Measured on TRN2 (axon), seed-0 inputs (cap=4): ~147.5 us HW exec,
rel err ~8e-4 vs the fp32 jax reference (bf16 tanh path dominates the
error).  Dense worst case (all lens 1023, cap=8): ~265 us.
"""

import os
from contextlib import ExitStack

import numpy as np

B, NQ, NKV, D, H = 8, 256, 1024, 256, 128
NCORES = 8
SLOT_KEYS = 128          # keys per slot
ACT_G = 32               # max keys per tanh group (one ACT instruction each)
VE_W = 264               # 256 V cols + 1 ones col + 7 pad cols
DEN_COL = 256            # index of the denominator column in VE / out

_prog_cache: dict[tuple, object] = {}

# kernel structure knobs (tuned on HW 2026-08-03; ~146us at cap=4)
CONFIG = {
    "bias_keys": 0,       # keys per slot via ACT fused-bias tanh (no DVE add)
    "copies": "dve",      # engine for PSUM->SBUF copies: "act" | "dve"
    "prefetch": True,     # emit slot s+1 loads/projections before slot s body
    "sums_bufs": 4,
    "th_bufs": 4,
    "act_g": 16,          # keys per grouped-tanh ACT instruction
    "taper": False,       # (only meaningful at act_g=32) shrink last groups
}


def _build_program(cap: int):
    """Build + compile the Bass program for `cap` slots per core."""
    import concourse.bass as bass  # noqa: F401  (registers engines)
    import concourse.tile as tile
    from concourse import bacc, mybir

    f32 = mybir.dt.float32
    bf16 = mybir.dt.bfloat16
    AF = mybir.ActivationFunctionType

    nc = bacc.Bacc("TRN2", target_bir_lowering=False, debug=False,
                   num_devices=NCORES)

    # DRAM I/O.  Layouts chosen so every DMA is contiguous per partition.
    kt = nc.dram_tensor("kt", [cap, 128, 2, 128], f32, kind="ExternalInput")
    qt = nc.dram_tensor("qt", [cap, 128, 2, 256], f32, kind="ExternalInput")
    ve = nc.dram_tensor("ve", [cap, 128, VE_W], f32, kind="ExternalInput")
    wqt = nc.dram_tensor("wqt", [128, 2, 128], f32, kind="ExternalInput")
    wkt = nc.dram_tensor("wkt", [128, 2, 128], f32, kind="ExternalInput")
    wvd = nc.dram_tensor("wvd", [128, 32, 32], bf16, kind="ExternalInput")
    out = nc.dram_tensor("out", [cap, 2, 128, VE_W], f32, kind="ExternalOutput")

    # Per-slot key schedule: groups of <=32 keys built by DVE adds + one
    # grouped tanh each; optionally the last BIAS_KEYS keys use ACT's fused
    # bias path (tanh(qp + kp_j) in one ACTIVATE, no DVE add) to balance
    # DVE and ACT.
    BIAS_KEYS = CONFIG["bias_keys"]
    gsz = CONFIG["act_g"]
    ndve = SLOT_KEYS - BIAS_KEYS
    groups = []
    j0 = 0
    while j0 < ndve:
        groups.append((j0, min(gsz, ndve - j0)))
        j0 += gsz
    if CONFIG.get("taper") and not BIAS_KEYS and ndve == 128 and gsz == 32:
        # Taper the slot's final groups so the add->tanh->matmul->exp->V
        # latency chain after the LAST DVE add is short (shrinks the
        # kernel tail where DVE sits idle).
        groups = [(0, 32), (32, 32), (64, 32), (96, 16), (112, 8), (120, 8)]

    with tile.TileContext(nc) as tc:
        with ExitStack() as ctx:
            consts = ctx.enter_context(tc.tile_pool(name="consts", bufs=1))
            kin = ctx.enter_context(tc.tile_pool(name="kin", bufs=2))
            qin = ctx.enter_context(tc.tile_pool(name="qin", bufs=2))
            vin = ctx.enter_context(tc.tile_pool(name="vin", bufs=2))
            proj = ctx.enter_context(tc.tile_pool(name="proj", bufs=2))
            sums_p = ctx.enter_context(
                tc.tile_pool(name="sums", bufs=CONFIG["sums_bufs"]))
            tanh_p = ctx.enter_context(
                tc.tile_pool(name="tanh", bufs=CONFIG["th_bufs"]))
            exp_p = ctx.enter_context(tc.tile_pool(name="expp", bufs=2))
            ps_proj = ctx.enter_context(
                tc.tile_pool(name="psproj", bufs=2, space="PSUM"))
            ps_sc = ctx.enter_context(
                tc.tile_pool(name="pssc", bufs=2, space="PSUM"))
            ps_out = ctx.enter_context(
                tc.tile_pool(name="psout", bufs=2, space="PSUM"))

            wqt_sb = consts.tile([128, 2, 128], f32)
            nc.sync.dma_start(out=wqt_sb[:], in_=wqt[:])
            wkt_sb = consts.tile([128, 2, 128], f32)
            nc.sync.dma_start(out=wkt_sb[:], in_=wkt[:])
            wvd_sb = consts.tile([128, 32, 32], bf16)
            nc.sync.dma_start(out=wvd_sb[:], in_=wvd[:])

            copy_eng = (nc.scalar.copy if CONFIG["copies"] == "act"
                        else nc.vector.tensor_copy)
            proj_copy = (nc.scalar.copy if CONFIG.get("proj_copies") == "act"
                         else copy_eng)

            def load_and_project(s):
                """DMA slot s inputs + compute kpT/qpT; returns SBUF tiles."""
                kt_sb = kin.tile([128, 2, 128], f32, tag="kt")
                nc.sync.dma_start(out=kt_sb[:], in_=kt[s])
                qt_sb = qin.tile([128, 2, 256], f32, tag="qt")
                nc.sync.dma_start(out=qt_sb[:], in_=qt[s])
                ve_sb = vin.tile([128, VE_W], f32, tag="ve")
                nc.sync.dma_start(out=ve_sb[:], in_=ve[s])

                # kpT[h, j] = sum_d Wk[h, d] K[j, d]  (contract d on partitions)
                kp_ps = ps_proj.tile([128, 128], f32, tag="kp")
                for c in range(2):
                    nc.tensor.matmul(kp_ps[:], wkt_sb[:, c, :], kt_sb[:, c, :],
                                     start=(c == 0), stop=(c == 1))
                kp_sb = proj.tile([128, 128], f32, tag="kp_sb")
                proj_copy(kp_sb[:], kp_ps[:])

                qp_ps = ps_proj.tile([128, 256], f32, tag="qp")
                for c in range(2):
                    nc.tensor.matmul(qp_ps[:], wqt_sb[:, c, :], qt_sb[:, c, :],
                                     start=(c == 0), stop=(c == 1))
                qp_sb = proj.tile([128, 256], bf16, tag="qp_sb")
                proj_copy(qp_sb[:], qp_ps[:])
                return kp_sb, qp_sb, ve_sb

            nxt = load_and_project(0)
            for s in range(cap):
                if not CONFIG["prefetch"] and s > 0:
                    nxt = load_and_project(s)
                kp_sb, qp_sb, ve_sb = nxt
                if CONFIG["prefetch"] and s + 1 < cap:
                    # software-pipeline: next slot's loads + projections are
                    # emitted first so each engine's FIFO has them before
                    # this slot's long tanh/add streams
                    nxt = load_and_project(s + 1)

                # scoresT[j, i] for this slot, built 32 rows at a time.
                sc_ps = ps_sc.tile([128, 256], f32, tag="sc")

                def score_mm(j, rhs):
                    sg, jl = divmod(j, 32)
                    nc.tensor.matmul(
                        sc_ps[sg * 32:(sg + 1) * 32, :],
                        wvd_sb[:, jl, :],
                        rhs,
                        start=(jl == 0), stop=(jl == 31),
                        tile_position=(0, sg * 32))

                if CONFIG.get("bias_mode", "tail") == "spread" and BIAS_KEYS:
                    # per 32-key score block: first (32-bpp) keys via DVE
                    # adds + one grouped tanh, last bpp keys via fused
                    # bias-tanh on ACT (spread evenly across the slot)
                    bpp = BIAS_KEYS // 4
                    for blk in range(4):
                        j0 = blk * 32
                        glen = 32 - bpp
                        sums = sums_p.tile([128, ACT_G, 256], bf16,
                                           tag="sums")
                        for jj in range(glen):
                            nc.vector.tensor_scalar_add(
                                out=sums[:, jj, :], in0=qp_sb[:],
                                scalar1=kp_sb[:, j0 + jj:j0 + jj + 1])
                        th = tanh_p.tile([128, ACT_G, 256], bf16, tag="th")
                        nc.scalar.activation(out=th[:, :glen, :],
                                             in_=sums[:, :glen, :],
                                             func=AF.Tanh)
                        for jj in range(glen):
                            score_mm(j0 + jj, th[:, jj, :])
                        thb = tanh_p.tile([128, max(bpp, 1), 256], bf16,
                                          tag="thb")
                        for bk in range(bpp):
                            j = j0 + glen + bk
                            nc.scalar.activation(out=thb[:, bk, :],
                                                 in_=qp_sb[:], func=AF.Tanh,
                                                 bias=kp_sb[:, j:j + 1])
                            score_mm(j, thb[:, bk, :])
                else:
                    for j0, glen in groups:
                        sums = sums_p.tile([128, ACT_G, 256], bf16,
                                           tag="sums")
                        for jj in range(glen):
                            nc.vector.tensor_scalar_add(
                                out=sums[:, jj, :], in0=qp_sb[:],
                                scalar1=kp_sb[:, j0 + jj:j0 + jj + 1])
                        th = tanh_p.tile([128, ACT_G, 256], bf16, tag="th")
                        nc.scalar.activation(out=th[:, :glen, :],
                                             in_=sums[:, :glen, :],
                                             func=AF.Tanh)
                        for jj in range(glen):
                            score_mm(j0 + jj, th[:, jj, :])

                    if BIAS_KEYS:
                        # tail keys: fused tanh(qp+kp_j) on ACT, no DVE add
                        thb = tanh_p.tile([128, BIAS_KEYS, 256], bf16,
                                          tag="thb")
                        for bk in range(BIAS_KEYS):
                            j = SLOT_KEYS - BIAS_KEYS + bk
                            nc.scalar.activation(out=thb[:, bk, :],
                                                 in_=qp_sb[:], func=AF.Tanh,
                                                 bias=kp_sb[:, j:j + 1])
                            score_mm(j, thb[:, bk, :])

                exp_sb = exp_p.tile([128, 256], f32, tag="exp")
                nc.scalar.activation(out=exp_sb[:], in_=sc_ps[:], func=AF.Exp)

                for ic in range(2):
                    o_ps = ps_out.tile([128, VE_W], f32, tag="o")
                    nc.tensor.matmul(o_ps[:],
                                     exp_sb[:, ic * 128:(ic + 1) * 128],
                                     ve_sb[:],
                                     start=True, stop=True)
                    o_sb = exp_p.tile([128, VE_W], f32, tag="o_sb")
                    copy_eng(o_sb[:], o_ps[:])
                    nc.sync.dma_start(out=out[s, ic], in_=o_sb[:])

    nc.compile()
    return nc


def _get_program(cap: int):
    key = (cap, tuple(sorted(CONFIG.items())))
    if key not in _prog_cache:
        _prog_cache[key] = _build_program(cap)
    return _prog_cache[key]


def _chunkT(a2d: np.ndarray, nfree: int) -> np.ndarray:
    """(n, 256) row-major -> (128, 2, n): [p, c, n] = a2d[n, 128c + p]."""
    return np.ascontiguousarray(
        a2d.T.reshape(2, 128, nfree).transpose(1, 0, 2))


def _prepare(Q_batch, K_batch, V_batch, valid_lens, Wq, Wk, Wv):
    Q = np.asarray(Q_batch, np.float32)
    K = np.asarray(K_batch, np.float32)
    V = np.asarray(V_batch, np.float32)
    L = np.asarray(valid_lens).astype(np.int64)
    Wq = np.asarray(Wq, np.float32)
    Wk = np.asarray(Wk, np.float32)
    Wv = np.asarray(Wv, np.float32)

    # Work list: one slot per 128-key block that contains any valid key.
    slots = []
    for b in range(B):
        nblk = max(1, int(-(-int(L[b]) // SLOT_KEYS)))
        nblk = min(nblk, NKV // SLOT_KEYS)
        for blk in range(nblk):
            slots.append((b, blk * SLOT_KEYS))
    cap = -(-len(slots) // NCORES)

    import ml_dtypes
    wqt = _chunkT(Wq, 128)
    wkt = _chunkT(Wk, 128)
    wvd = np.zeros((128, 32, 32), np.float32)
    wvd[:, np.arange(32), np.arange(32)] = Wv[:, None]
    wvd = wvd.astype(ml_dtypes.bfloat16)

    qts = [_chunkT(Q[b], 256) for b in range(B)]

    in_maps = []
    core_slots = []
    for c in range(NCORES):
        items = slots[c * cap:(c + 1) * cap]
        core_slots.append(items)
        kt_arr = np.zeros((cap, 128, 2, 128), np.float32)
        qt_arr = np.zeros((cap, 128, 2, 256), np.float32)
        ve_arr = np.zeros((cap, 128, VE_W), np.float32)
        for si, (b, j0) in enumerate(items):
            kt_arr[si] = _chunkT(K[b, j0:j0 + SLOT_KEYS], SLOT_KEYS)
            qt_arr[si] = qts[b]
            nval = int(np.clip(int(L[b]) - j0, 0, SLOT_KEYS))
            ve_arr[si, :nval, :256] = V[b, j0:j0 + nval]
            ve_arr[si, :nval, DEN_COL] = 1.0
        in_maps.append({
            "kt": kt_arr, "qt": qt_arr, "ve": ve_arr,
            "wqt": wqt, "wkt": wkt, "wvd": wvd,
        })
    return cap, core_slots, in_maps


def _gather(core_slots, results) -> np.ndarray:
    acc = np.zeros((B, NQ, 257), np.float64)
    for c, items in enumerate(core_slots):
        o = results[c]["out"]  # (cap, 2, 128, VE_W)
        for si, (b, _j0) in enumerate(items):
            part = o[si].reshape(NQ, VE_W)[:, :257]
            acc[b] += part
    return (acc[:, :, :256] / acc[:, :, 256:257]).astype(np.float32)


def _install_ntff_hook():
    """Register the axon NTFF profile hook that bass_utils reads via
    antenv.axon_hooks (the shipped antenv stub lacks that module)."""
    import contextlib
    import ctypes
    import sys
    import types

    try:
        from antenv.axon_hooks import get_axon_ntff_profile_hook
        if get_axon_ntff_profile_hook() is not None:
            return
    except ImportError:
        pass

    so_path = "/opt/axon/libaxon_pjrt.so"
    if not os.path.exists(so_path):
        return
    lib = ctypes.CDLL(so_path)
    if not hasattr(lib, "axon_start_nrt_profile"):
        return
    lib.axon_start_nrt_profile.argtypes = [
        ctypes.POINTER(ctypes.c_int64), ctypes.c_size_t]
    lib.axon_start_nrt_profile.restype = ctypes.c_int64
    lib.axon_stop_nrt_profile.argtypes = [ctypes.c_char_p]
    lib.axon_stop_nrt_profile.restype = ctypes.c_int64

    @contextlib.contextmanager
    def _hook(output_dir, device_ids):
        import jax
        jax.devices()
        if device_ids:
            ids = (ctypes.c_int64 * len(device_ids))(*device_ids)
            rc = lib.axon_start_nrt_profile(ids, len(device_ids))
        else:
            rc = lib.axon_start_nrt_profile(None, 0)
        if rc != 0:
            raise RuntimeError(f"axon_start_nrt_profile rc={rc}")
        try:
            yield
        finally:
            n = lib.axon_stop_nrt_profile(str(output_dir).encode())
            print(f"ntff profile: {n} file(s) written to {output_dir}")

    mod = types.ModuleType("antenv.axon_hooks")
    mod.get_axon_ntff_profile_hook = lambda: _hook
    mod.set_axon_ntff_profile_hook = lambda h: None
    sys.modules["antenv.axon_hooks"] = mod
    import antenv
    antenv.axon_hooks = mod


def run(Q_batch, K_batch, V_batch, valid_lens, Wq, Wk, Wv,
        trace: bool = False):
    """Returns (output, exec_time_ns_or_None)."""
    from concourse.bass_utils import run_bass_kernel_spmd

    if trace:
        _install_ntff_hook()

    cap, core_slots, in_maps = _prepare(
        Q_batch, K_batch, V_batch, valid_lens, Wq, Wk, Wv)
    nc = _get_program(cap)

    if os.environ.get("ADD_ATTN_SIM"):
        from concourse.bass_interp import CoreSim
        ncores = int(os.environ.get("ADD_ATTN_SIM_CORES", NCORES))
        results = []
        for c in range(ncores):
            sim = CoreSim(nc)
            for name, arr in in_maps[c].items():
                sim.tensor(name)[:] = arr
            sim.simulate()
            results.append({"out": np.array(sim.tensor("out"))})
        core_slots = core_slots[:ncores]
        return _gather(core_slots, results), None

    res = run_bass_kernel_spmd(nc, in_maps, core_ids=list(range(NCORES)),
                               trace=trace)
    return _gather(core_slots, res.results), res.exec_time_ns


def kernel(Q_batch, K_batch, V_batch, valid_lens, Wq, Wk, Wv):
    out, _ = run(Q_batch, K_batch, V_batch, valid_lens, Wq, Wk, Wv)
    return out



# revision 10
# speedup vs baseline: 3.1754x; 3.1754x over previous
"""Additive (Bahdanau) attention on TRN2, one batch per core, SPMD over 8.

Math per batch (Q (256,256), K (1024,256), V (1024,256), H=128):
    qp = Q @ Wq.T ; kp = K @ Wk.T
    s[i,j] = sum_h Wv[h] * tanh(qp[i,h] + kp[j,h])
    out    = softmax_j(s, masked) @ V

The O(NQ*NKV*H) tanh is replaced by a 3-term sine expansion fitted to tanh
(density-weighted LSQ, wrms 7.7e-3 over the actual argument distribution):

    tanh(x) ~ b1 sin(F x) + b2 sin(3F x) + b3 sin(6F x),  F = 0.3655

sin(w(a+b)) = sin(wa)cos(wb) + cos(wa)sin(wb) makes the scores SEPARABLE:
one PE matmul with contraction 6*H = 768 instead of 33M elementwise tanh
per core.  Base features sin/cos(F*x) come from the ACT Sin table (|arg| <=
F*4.21 + pi/2 = 3.11 < pi, the table's hard valid range); the 3F and 6F
harmonics are built algebraically on DVE with fused scalar_tensor_tensor:
    s3 = (3 - 4 s1^2) s1 ; c3 = (4 c1^2 - 3) c1 ; s6 = 2 s3 c3 ; c6 = 1 - 2 s3^2
The k-side uses c6-1 (the +1 adds a per-query row constant, which softmax
cancels); the q-side folds the +1 into its coef scaling op.

Softmax uses no max-subtraction (|s| <= sum|b_m|*sum|Wv| ~ 6, exp is safe);
masked keys are handled by zeroing their V rows and ones-column on the host,
so partial numerator/denominator sums are exact.  Division happens on host.

Measured: see test.py (HW exec ~O(10us) vs 144.5us for the elementwise
baseline); rel err ~5e-3 vs the fp32 jax reference.
"""

import os
from contextlib import ExitStack

import numpy as np

B, NQ, NKV, D, H = 8, 256, 1024, 256, 128
NCORES = 8
VW = 264                 # V cols (256) + ones col (1) + pad to 264
F = 0.3655
B3 = (1.171000692830541, 0.3125350842862747, 0.0884505512829242)

_prog_cache: dict[tuple, object] = {}


def _build_program():
    import concourse.bass as bass  # noqa: F401  (registers engines)
    import concourse.tile as tile
    from concourse import bacc, mybir

    f32 = mybir.dt.float32
    bf16 = mybir.dt.bfloat16
    AF = mybir.ActivationFunctionType
    ALU = mybir.AluOpType

    nc = bacc.Bacc("TRN2", target_bir_lowering=False, debug=False,
                   num_devices=NCORES)

    qt = nc.dram_tensor("qt", [128, 2, 256], bf16, kind="ExternalInput")
    kt = nc.dram_tensor("kt", [2, 128, 2, 512], bf16, kind="ExternalInput")
    vv = nc.dram_tensor("vv", [128, 8, VW], bf16, kind="ExternalInput")
    wqt = nc.dram_tensor("wqt", [128, 2, 128], bf16, kind="ExternalInput")
    wkt = nc.dram_tensor("wkt", [128, 2, 128], bf16, kind="ExternalInput")
    coef = nc.dram_tensor("coef", [128, 3], f32, kind="ExternalInput")
    # runtime base frequency (col0=+F, col1=-F) so the sin args stay in
    # the ACT table's [-pi, pi] range for ANY input magnitudes
    fsc = nc.dram_tensor("fsc", [128, 2], f32, kind="ExternalInput")
    out = nc.dram_tensor("out", [2, 128, VW], f32, kind="ExternalOutput")

    HPI = float(np.pi / 2)

    with tile.TileContext(nc) as tc:
        with ExitStack() as ctx:
            consts = ctx.enter_context(tc.tile_pool(name="consts", bufs=1))
            sb = ctx.enter_context(tc.tile_pool(name="sb", bufs=1))
            ps_qp = ctx.enter_context(
                tc.tile_pool(name="psqp", bufs=1, space="PSUM"))
            ps_kp = ctx.enter_context(
                tc.tile_pool(name="pskp", bufs=1, space="PSUM"))
            ps_sc = ctx.enter_context(
                tc.tile_pool(name="pssc", bufs=3, space="PSUM"))
            ps_o = ctx.enter_context(
                tc.tile_pool(name="pso", bufs=1, space="PSUM"))

            # constants + ACT table warmups (sin table loads under the DMAs)
            halfpi = consts.tile([128, 1], f32)
            nc.gpsimd.memset(halfpi[:], HPI)
            nhalfpi = consts.tile([128, 1], f32)
            nc.gpsimd.memset(nhalfpi[:], -HPI)
            dummy = consts.tile([128, 1], f32)
            nc.scalar.activation(out=dummy[:], in_=halfpi[:], func=AF.Sin)

            wqt_sb = consts.tile([128, 2, 128], bf16)
            nc.sync.dma_start(out=wqt_sb[:], in_=wqt[:])
            qt_sb = consts.tile([128, 2, 256], bf16)
            nc.sync.dma_start(out=qt_sb[:], in_=qt[:])
            wkt_sb = consts.tile([128, 2, 128], bf16)
            nc.sync.dma_start(out=wkt_sb[:], in_=wkt[:])
            kt_sb = [consts.tile([128, 2, 512], bf16, tag=f"kt{jh}",
                                 name=f"kt_sb{jh}")
                     for jh in range(2)]
            for jh in range(2):
                nc.sync.dma_start(out=kt_sb[jh][:], in_=kt[jh])
            coef_sb = consts.tile([128, 3], f32)
            nc.sync.dma_start(out=coef_sb[:], in_=coef[:])
            fsc_sb = consts.tile([128, 2], f32)
            nc.sync.dma_start(out=fsc_sb[:], in_=fsc[:])
            vv_sb = consts.tile([128, 8, VW], bf16)
            nc.sync.dma_start(out=vv_sb[:], in_=vv[:])

            # ---- projections (PE): qp/kp with h on partitions -------------
            qp_ps = ps_qp.tile([128, 256], f32)
            for c in range(2):
                nc.tensor.matmul(qp_ps[:], wqt_sb[:, c, :], qt_sb[:, c, :],
                                 start=(c == 0), stop=(c == 1))
            kp_ps = []
            for jh in range(2):
                kp = ps_kp.tile([128, 512], f32, tag=f"kp{jh}")
                for c in range(2):
                    nc.tensor.matmul(kp[:], wkt_sb[:, c, :],
                                     kt_sb[jh][:, c, :],
                                     start=(c == 0), stop=(c == 1))
                kp_ps.append(kp)

            # ---- base features (ACT Sin, args within [-pi, pi]) -----------
            # A_q = [s1q | -c1q]   (256+256)
            a_q = sb.tile([128, 512], bf16)
            nc.scalar.activation(out=a_q[:, 0:256], in_=qp_ps[:],
                                 func=AF.Sin, scale=fsc_sb[:, 0:1])
            nc.scalar.activation(out=a_q[:, 256:512], in_=qp_ps[:],
                                 func=AF.Sin, scale=fsc_sb[:, 1:2],
                                 bias=nhalfpi[:])
            # A_k[jh] = [-s1k | c1k]   (512+512)
            a_k = []
            for jh in range(2):
                ak = sb.tile([128, 1024], bf16, tag=f"ak{jh}")
                nc.scalar.activation(out=ak[:, 0:512], in_=kp_ps[jh][:],
                                     func=AF.Sin, scale=fsc_sb[:, 1:2])
                nc.scalar.activation(out=ak[:, 512:1024], in_=kp_ps[jh][:],
                                     func=AF.Sin, scale=fsc_sb[:, 0:1],
                                     bias=halfpi[:])
                a_k.append(ak)
            # trigger the exp table load now (hides under DVE chains)
            dummy2 = consts.tile([128, 1], f32)
            nc.scalar.activation(out=dummy2[:], in_=halfpi[:], func=AF.Exp)

            # ---- harmonic chains (DVE scalar_tensor_tensor, bf16) ---------
            stt = nc.vector.scalar_tensor_tensor
            # q-side: T=(A*-4)*A ; SC3=(T+3)*A -> [s3q | c3q]
            t_q = sb.tile([128, 512], bf16)
            stt(out=t_q[:], in0=a_q[:], scalar=-4.0, in1=a_q[:],
                op0=ALU.mult, op1=ALU.mult)
            sc3_q = sb.tile([128, 512], bf16)
            stt(out=sc3_q[:], in0=t_q[:], scalar=3.0, in1=a_q[:],
                op0=ALU.add, op1=ALU.mult)
            s6_q = sb.tile([128, 256], bf16)
            stt(out=s6_q[:], in0=sc3_q[:, 0:256], scalar=2.0,
                in1=sc3_q[:, 256:512], op0=ALU.mult, op1=ALU.mult)
            c6_q = sb.tile([128, 256], bf16)   # = cos6q - 1
            stt(out=c6_q[:], in0=sc3_q[:, 0:256], scalar=-2.0,
                in1=sc3_q[:, 0:256], op0=ALU.mult, op1=ALU.mult)

            # q-feature scaling by coef columns (signs pre-folded on host)
            ts = nc.vector.tensor_scalar
            fq01 = sb.tile([128, 512], bf16)   # [s1q*b1Wv | -c1q*b1Wv]
            ts(out=fq01[:], in0=a_q[:], scalar1=coef_sb[:, 0:1], scalar2=None,
               op0=ALU.mult)
            fq23 = sb.tile([128, 512], bf16)   # [-s3q*b3Wv | -c3q*b3Wv]
            ts(out=fq23[:], in0=sc3_q[:], scalar1=coef_sb[:, 1:2],
               scalar2=None, op0=ALU.mult)
            fq4 = sb.tile([128, 256], bf16)    # s6q*b6Wv
            ts(out=fq4[:], in0=s6_q[:], scalar1=coef_sb[:, 2:3], scalar2=None,
               op0=ALU.mult)
            fq5 = sb.tile([128, 256], bf16)    # (c6q'+1)*b6Wv = cos6q*b6Wv
            ts(out=fq5[:], in0=c6_q[:], scalar1=1.0, scalar2=coef_sb[:, 2:3],
               op0=ALU.add, op1=ALU.mult)

            # k-side chains per half
            sc3_k, s6_k, c6_k = [], [], []
            for jh in range(2):
                t_k = sb.tile([128, 1024], bf16, tag=f"tk{jh}")
                stt(out=t_k[:], in0=a_k[jh][:], scalar=-4.0, in1=a_k[jh][:],
                    op0=ALU.mult, op1=ALU.mult)
                sc3 = sb.tile([128, 1024], bf16, tag=f"sc3k{jh}")
                stt(out=sc3[:], in0=t_k[:], scalar=3.0, in1=a_k[jh][:],
                    op0=ALU.add, op1=ALU.mult)   # [-s3k | -c3k]
                s6 = sb.tile([128, 512], bf16, tag=f"s6k{jh}")
                stt(out=s6[:], in0=sc3[:, 0:512], scalar=2.0,
                    in1=sc3[:, 512:1024], op0=ALU.mult, op1=ALU.mult)
                c6 = sb.tile([128, 512], bf16, tag=f"c6k{jh}")  # cos6k - 1
                stt(out=c6[:], in0=sc3[:, 0:512], scalar=-2.0,
                    in1=sc3[:, 0:512], op0=ALU.mult, op1=ALU.mult)
                sc3_k.append(sc3)
                s6_k.append(s6)
                c6_k.append(c6)

            # ---- scores (PE) + exp (ACT) ----------------------------------
            def fk_slices(jc):
                jh, l = divmod(jc, 4)
                lo, hi = l * 128, (l + 1) * 128
                return [
                    a_k[jh][:, 512 + lo:512 + hi],    # c1k
                    a_k[jh][:, lo:hi],                # -s1k
                    sc3_k[jh][:, 512 + lo:512 + hi],  # -c3k
                    sc3_k[jh][:, lo:hi],              # -s3k
                    c6_k[jh][:, lo:hi],               # cos6k - 1
                    s6_k[jh][:, lo:hi],               # s6k
                ]

            fq_list = [fq01[:, 0:256], fq01[:, 256:512],
                       fq23[:, 0:256], fq23[:, 256:512],
                       fq4[:], fq5[:]]

            ex = []
            for pr in range(4):
                sc_ps = ps_sc.tile([128, 512], f32, tag="sc")
                for half in range(2):
                    jc = pr * 2 + half
                    fks = fk_slices(jc)
                    o = sc_ps[:, half * 256:(half + 1) * 256]
                    for f in range(6):
                        nc.tensor.matmul(o, fks[f], fq_list[f],
                                         start=(f == 0), stop=(f == 5))
                e = sb.tile([128, 512], bf16, tag=f"ex{pr}")
                nc.scalar.activation(out=e[:], in_=sc_ps[:], func=AF.Exp)
                ex.append(e)

            # ---- numerator/denominator (PE) + writeback -------------------
            for ic in range(2):
                o_ps = ps_o.tile([128, VW], f32, tag=f"o{ic}")
                for jc in range(8):
                    pr, half = divmod(jc, 2)
                    lo = half * 256 + ic * 128
                    nc.tensor.matmul(o_ps[:], ex[pr][:, lo:lo + 128],
                                     vv_sb[:, jc, :],
                                     start=(jc == 0), stop=(jc == 7))
                o_sb = sb.tile([128, VW], f32, tag=f"osb{ic}")
                nc.scalar.copy(o_sb[:], o_ps[:])
                nc.sync.dma_start(out=out[ic], in_=o_sb[:])

    nc.compile()
    return nc


def _get_program():
    if "p" not in _prog_cache:
        _prog_cache["p"] = _build_program()
    return _prog_cache["p"]


def _chunkT(a2d: np.ndarray, nfree: int) -> np.ndarray:
    """(n, 256) row-major -> (128, 2, n): [p, c, n] = a2d[n, 128c + p]."""
    return np.ascontiguousarray(
        a2d.T.reshape(2, 128, nfree).transpose(1, 0, 2))


def _fit_b(F: float, sig: float, xlim: float) -> np.ndarray:
    """Density-weighted LSQ of tanh(x) ~ b1 sin(Fx)+b2 sin(3Fx)+b3 sin(6Fx)."""
    x = np.linspace(0.0, xlim, 3001)
    w = np.sqrt(np.exp(-x ** 2 / (2.0 * sig * sig)) + 2e-6)
    A = np.stack([np.sin(F * x), np.sin(3 * F * x), np.sin(6 * F * x)], 1)
    b, *_ = np.linalg.lstsq(A * w[:, None], np.tanh(x) * w, rcond=None)
    return b


def _prepare(Q_batch, K_batch, V_batch, valid_lens, Wq, Wk, Wv):
    import ml_dtypes
    BF = ml_dtypes.bfloat16

    Q = np.asarray(Q_batch, np.float32)
    K = np.asarray(K_batch, np.float32)
    V = np.asarray(V_batch, np.float32)
    L = np.asarray(valid_lens).astype(np.int64)
    Wq = np.asarray(Wq, np.float32)
    Wk = np.asarray(Wk, np.float32)
    Wv = np.asarray(Wv, np.float32)

    wqt = _chunkT(Wq, 128).astype(BF)
    wkt = _chunkT(Wk, 128).astype(BF)
    Qb = Q.astype(BF).astype(np.float32)
    Kb = K.astype(BF).astype(np.float32)
    Wqb = Wq.astype(BF).astype(np.float32)
    Wkb = Wk.astype(BF).astype(np.float32)

    in_maps = []
    for b in range(B):
        qt = _chunkT(Q[b], 256).astype(BF)
        kt = np.stack([_chunkT(K[b, jh * 512:(jh + 1) * 512], 512)
                       for jh in range(2)]).astype(BF)
        n = int(L[b])
        vr = np.zeros((NKV, VW), np.float32)
        vr[:n, :256] = V[b, :n]
        vr[:n, 256] = 1.0
        vvb = np.ascontiguousarray(
            vr.reshape(8, 128, VW).transpose(1, 0, 2)).astype(BF)

        # per-core adaptive base frequency: the device computes the same
        # qp/kp (bf16 products, fp32 accum); keep F*xmax + pi/2 <= pi-0.03
        qp = Qb[b] @ Wqb.T
        kp = Kb[b] @ Wkb.T
        xmax = float(max(np.abs(qp).max(), np.abs(kp).max()))
        Fb = min(F, (np.pi / 2 - 0.03) / max(xmax, 1e-6))
        sig = float(np.sqrt(qp.std() ** 2 + kp.std() ** 2))
        xlim = float(np.abs(qp).max() + np.abs(kp).max()) + 0.3
        bfit = _fit_b(Fb, max(sig, 1e-3), xlim)
        coef = np.stack([bfit[0] * Wv, -bfit[1] * Wv,
                         bfit[2] * Wv], 1).astype(np.float32)
        fsc = np.tile(np.array([Fb, -Fb], np.float32), (128, 1))
        in_maps.append({"qt": qt, "kt": kt, "vv": vvb,
                        "wqt": wqt, "wkt": wkt, "coef": coef, "fsc": fsc})
    return in_maps


def _gather(results) -> np.ndarray:
    outp = np.zeros((B, NQ, 256), np.float32)
    for b in range(B):
        o = results[b]["out"].astype(np.float64)  # (2, 128, VW)
        num = o[:, :, :256].reshape(NQ, 256)
        den = o[:, :, 256].reshape(NQ, 1)
        outp[b] = (num / den).astype(np.float32)
    return outp


def _install_ntff_hook():
    """Register the axon NTFF profile hook that bass_utils reads via
    antenv.axon_hooks (the shipped antenv stub lacks that module)."""
    import contextlib
    import ctypes
    import sys
    import types

    try:
        from antenv.axon_hooks import get_axon_ntff_profile_hook
        if get_axon_ntff_profile_hook() is not None:
            return
    except ImportError:
        pass

    so_path = "/opt/axon/libaxon_pjrt.so"
    if not os.path.exists(so_path):
        return
    lib = ctypes.CDLL(so_path)
    if not hasattr(lib, "axon_start_nrt_profile"):
        return
    lib.axon_start_nrt_profile.argtypes = [
        ctypes.POINTER(ctypes.c_int64), ctypes.c_size_t]
    lib.axon_start_nrt_profile.restype = ctypes.c_int64
    lib.axon_stop_nrt_profile.argtypes = [ctypes.c_char_p]
    lib.axon_stop_nrt_profile.restype = ctypes.c_int64

    @contextlib.contextmanager
    def _hook(output_dir, device_ids):
        import jax
        jax.devices()
        if device_ids:
            ids = (ctypes.c_int64 * len(device_ids))(*device_ids)
            rc = lib.axon_start_nrt_profile(ids, len(device_ids))
        else:
            rc = lib.axon_start_nrt_profile(None, 0)
        if rc != 0:
            raise RuntimeError(f"axon_start_nrt_profile rc={rc}")
        try:
            yield
        finally:
            n = lib.axon_stop_nrt_profile(str(output_dir).encode())
            print(f"ntff profile: {n} file(s) written to {output_dir}")

    mod = types.ModuleType("antenv.axon_hooks")
    mod.get_axon_ntff_profile_hook = lambda: _hook
    mod.set_axon_ntff_profile_hook = lambda h: None
    sys.modules["antenv.axon_hooks"] = mod
    import antenv
    antenv.axon_hooks = mod


def run(Q_batch, K_batch, V_batch, valid_lens, Wq, Wk, Wv,
        trace: bool = False):
    """Returns (output, exec_time_ns_or_None)."""
    from concourse.bass_utils import run_bass_kernel_spmd

    if trace:
        _install_ntff_hook()

    in_maps = _prepare(Q_batch, K_batch, V_batch, valid_lens, Wq, Wk, Wv)
    nc = _get_program()

    if os.environ.get("ADD_ATTN_SIM"):
        from concourse.bass_interp import CoreSim
        ncores = int(os.environ.get("ADD_ATTN_SIM_CORES", NCORES))
        results = []
        for c in range(ncores):
            sim = CoreSim(nc)
            for name, arr in in_maps[c].items():
                sim.tensor(name)[:] = arr
            sim.simulate()
            results.append({"out": np.array(sim.tensor("out"))})
        results += [{"out": np.ones((2, 128, VW), np.float32)}
                    ] * (NCORES - ncores)
        return _gather(results), None

    res = run_bass_kernel_spmd(nc, in_maps, core_ids=list(range(NCORES)),
                               trace=trace)
    return _gather(res.results), res.exec_time_ns


def kernel(Q_batch, K_batch, V_batch, valid_lens, Wq, Wk, Wv):
    out, _ = run(Q_batch, K_batch, V_batch, valid_lens, Wq, Wk, Wv)
    return out


# revision 15
# speedup vs baseline: 3.7421x; 1.1784x over previous
"""Additive (Bahdanau) attention on TRN2, one batch per core, SPMD over 8.

Math per batch (Q (256,256), K (1024,256), V (1024,256), H=128):
    qp = Q @ Wq.T ; kp = K @ Wk.T
    s[i,j] = sum_h Wv[h] * tanh(qp[i,h] + kp[j,h])
    out    = softmax_j(s, masked) @ V

The O(NQ*NKV*H) tanh is replaced by a 3-term sine expansion fitted to tanh
(density-weighted LSQ, wrms 7.7e-3 over the actual argument distribution):

    tanh(x) ~ b1 sin(F x) + b2 sin(3F x) + b3 sin(6F x),  F = 0.3655

sin(w(a+b)) = sin(wa)cos(wb) + cos(wa)sin(wb) makes the scores SEPARABLE:
one PE matmul with contraction 6*H = 768 instead of 33M elementwise tanh
per core.  Base features sin/cos(F*x) come from the ACT Sin table (|arg| <=
F*4.21 + pi/2 = 3.11 < pi, the table's hard valid range); the 3F and 6F
harmonics are built algebraically on DVE with fused scalar_tensor_tensor:
    s3 = (3 - 4 s1^2) s1 ; c3 = (4 c1^2 - 3) c1 ; s6 = 2 s3 c3 ; c6 = 1 - 2 s3^2
The k-side uses c6-1 (the +1 adds a per-query row constant, which softmax
cancels); the q-side folds the +1 into its coef scaling op.

Softmax uses no max-subtraction (|s| <= sum|b_m|*sum|Wv| ~ 6, exp is safe);
masked keys are handled by zeroing their V rows and ones-column on the host,
so partial numerator/denominator sums are exact.  Division happens on host.

Measured: see test.py (HW exec ~O(10us) vs 144.5us for the elementwise
baseline); rel err ~5e-3 vs the fp32 jax reference.
"""

import os
from contextlib import ExitStack

import numpy as np

B, NQ, NKV, D, H = 8, 256, 1024, 256, 128
NCORES = 8
VW = 264                 # V cols (256) + ones col (1) + pad to 264
F = 0.3655
B3 = (1.171000692830541, 0.3125350842862747, 0.0884505512829242)

_prog_cache: dict[tuple, object] = {}


def _build_program():
    import concourse.bass as bass  # noqa: F401  (registers engines)
    import concourse.tile as tile
    from concourse import bacc, mybir

    f32 = mybir.dt.float32
    bf16 = mybir.dt.bfloat16
    AF = mybir.ActivationFunctionType
    ALU = mybir.AluOpType

    nc = bacc.Bacc("TRN2", target_bir_lowering=False, debug=False,
                   num_devices=NCORES)

    # qwk = [qt(512) | wqt(256) | wkt(256)] packed for one early DMA
    qwk = nc.dram_tensor("qwk", [128, 1024], bf16, kind="ExternalInput")
    kt = nc.dram_tensor("kt", [2, 128, 2, 512], bf16, kind="ExternalInput")
    vv = nc.dram_tensor("vv", [128, 8, VW], bf16, kind="ExternalInput")
    # fcoef cols: 0:+F 1:-F 2:+pi/2 3:-pi/2 4:b1*Wv 5:-b2*Wv 6:-4b3*Wv 7:2b3*Wv
    fcoef = nc.dram_tensor("fcoef", [128, 8], f32, kind="ExternalInput")
    out = nc.dram_tensor("out", [2, 128, VW], f32, kind="ExternalOutput")

    with tile.TileContext(nc) as tc:
        with ExitStack() as ctx:
            consts = ctx.enter_context(tc.tile_pool(name="consts", bufs=1))
            sb = ctx.enter_context(tc.tile_pool(name="sb", bufs=1))
            ps_qp = ctx.enter_context(
                tc.tile_pool(name="psqp", bufs=1, space="PSUM"))
            ps_kp = ctx.enter_context(
                tc.tile_pool(name="pskp", bufs=1, space="PSUM"))
            ps_sc = ctx.enter_context(
                tc.tile_pool(name="pssc", bufs=3, space="PSUM"))
            ps_o = ctx.enter_context(
                tc.tile_pool(name="pso", bufs=1, space="PSUM"))

            # spread DMA descriptor issue across idle engines
            fc_sb = consts.tile([128, 8], f32)
            nc.scalar.dma_start(out=fc_sb[:], in_=fcoef[:])
            kt_sb = [consts.tile([128, 2, 512], bf16, tag=f"kt{jh}",
                                 name=f"kt_sb{jh}")
                     for jh in range(2)]
            nc.sync.dma_start(out=kt_sb[0][:], in_=kt[0])
            qwk_sb = consts.tile([128, 1024], bf16)
            nc.gpsimd.dma_start(out=qwk_sb[:], in_=qwk[:])
            nc.sync.dma_start(out=kt_sb[1][:], in_=kt[1])
            vv_sb = consts.tile([128, 8, VW], bf16)
            nc.gpsimd.dma_start(out=vv_sb[:], in_=vv[:])

            # sin table preload (hides under the DMAs; fc_sb col2 = pi/2)
            dummy = consts.tile([128, 1], f32)
            nc.scalar.activation(out=dummy[:], in_=fc_sb[:, 2:3], func=AF.Sin)

            def qt_c(c):
                return qwk_sb[:, c * 256:(c + 1) * 256]

            def wqt_c(c):
                return qwk_sb[:, 512 + c * 128:512 + (c + 1) * 128]

            def wkt_c(c):
                return qwk_sb[:, 768 + c * 128:768 + (c + 1) * 128]

            # ---- projections (PE): qp/kp with h on partitions -------------
            kp_ps = []
            for jh in range(2):
                kp = ps_kp.tile([128, 512], f32, tag=f"kp{jh}",
                                name=f"kp{jh}")
                for c in range(2):
                    nc.tensor.matmul(kp[:], wkt_c(c), kt_sb[jh][:, c, :],
                                     start=(c == 0), stop=(c == 1))
                kp_ps.append(kp)
                if jh == 0:
                    qp_ps = ps_qp.tile([128, 256], f32)
                    for c in range(2):
                        nc.tensor.matmul(qp_ps[:], wqt_c(c), qt_c(c),
                                         start=(c == 0), stop=(c == 1))

            # ---- base features (ACT Sin, args within [-pi, pi]) -----------
            # A_k[jh] = [-s1k | c1k]   (512+512)
            a_k = []
            for jh in range(2):
                ak = sb.tile([128, 1024], bf16, tag=f"ak{jh}",
                             name=f"ak{jh}")
                nc.scalar.activation(out=ak[:, 0:512], in_=kp_ps[jh][:],
                                     func=AF.Sin, scale=fc_sb[:, 1:2])
                nc.scalar.activation(out=ak[:, 512:1024], in_=kp_ps[jh][:],
                                     func=AF.Sin, scale=fc_sb[:, 0:1],
                                     bias=fc_sb[:, 2:3])
                a_k.append(ak)
                if jh == 0:
                    # A_q = [s1q | -c1q]   (256+256)
                    a_q = sb.tile([128, 512], bf16)
                    nc.scalar.activation(out=a_q[:, 0:256], in_=qp_ps[:],
                                         func=AF.Sin, scale=fc_sb[:, 0:1])
                    nc.scalar.activation(out=a_q[:, 256:512], in_=qp_ps[:],
                                         func=AF.Sin, scale=fc_sb[:, 1:2],
                                         bias=fc_sb[:, 3:4])

            # ---- harmonic chains (DVE: TT gets bf16 2x, TS gets 4x) -------
            tt = nc.vector.tensor_tensor
            ts = nc.vector.tensor_scalar

            # k-side jh0 first (scores j-chunks 0-3 depend on it)
            sc3_k, s6_k, c6_k = [{} for _ in range(3)]

            def k_chain(jh):
                ak = a_k[jh]
                t_k = sb.tile([128, 1024], bf16, tag=f"tk{jh}",
                              name=f"tk{jh}")
                tt(out=t_k[:], in0=ak[:], in1=ak[:], op=ALU.mult)
                u_k = sb.tile([128, 1024], bf16, tag=f"uk{jh}",
                              name=f"uk{jh}")
                ts(out=u_k[:], in0=t_k[:], scalar1=-4.0, scalar2=3.0,
                   op0=ALU.mult, op1=ALU.add)
                sc3 = sb.tile([128, 1024], bf16, tag=f"sc3k{jh}",
                              name=f"sc3k{jh}")
                tt(out=sc3[:], in0=u_k[:], in1=ak[:], op=ALU.mult)
                # [-s3k | -c3k]
                s6 = sb.tile([128, 512], bf16, tag=f"s6k{jh}",
                             name=f"s6k{jh}")
                tt(out=s6[:], in0=sc3[:, 0:512], in1=sc3[:, 512:1024],
                   op=ALU.mult)          # = s3k*c3k
                c6 = sb.tile([128, 512], bf16, tag=f"c6k{jh}",
                             name=f"c6k{jh}")
                tt(out=c6[:], in0=sc3[:, 0:512], in1=sc3[:, 0:512],
                   op=ALU.mult)          # = s3k^2
                sc3_k[jh], s6_k[jh], c6_k[jh] = sc3, s6, c6

            k_chain(0)

            # q-side chain + feature scaling
            t_q = sb.tile([128, 512], bf16)
            tt(out=t_q[:], in0=a_q[:], in1=a_q[:], op=ALU.mult)
            u_q = sb.tile([128, 512], bf16)
            ts(out=u_q[:], in0=t_q[:], scalar1=-4.0, scalar2=3.0,
               op0=ALU.mult, op1=ALU.add)
            sc3_q = sb.tile([128, 512], bf16)      # [s3q | c3q]
            tt(out=sc3_q[:], in0=u_q[:], in1=a_q[:], op=ALU.mult)
            s6_q = sb.tile([128, 256], bf16)       # s3q*c3q
            tt(out=s6_q[:], in0=sc3_q[:, 0:256], in1=sc3_q[:, 256:512],
               op=ALU.mult)
            c6_q = sb.tile([128, 256], bf16)       # s3q^2
            tt(out=c6_q[:], in0=sc3_q[:, 0:256], in1=sc3_q[:, 0:256],
               op=ALU.mult)

            fq01 = sb.tile([128, 512], bf16)   # [s1q | -c1q] * b1Wv
            ts(out=fq01[:], in0=a_q[:], scalar1=fc_sb[:, 4:5], scalar2=None,
               op0=ALU.mult)
            fq23 = sb.tile([128, 512], bf16)   # [s3q | c3q] * (-b2Wv)
            ts(out=fq23[:], in0=sc3_q[:], scalar1=fc_sb[:, 5:6],
               scalar2=None, op0=ALU.mult)
            fq4 = sb.tile([128, 256], bf16)    # s3q c3q * (-4 b3Wv)
            ts(out=fq4[:], in0=s6_q[:], scalar1=fc_sb[:, 6:7], scalar2=None,
               op0=ALU.mult)
            fq5 = sb.tile([128, 256], bf16)    # s3q^2*(-4b3Wv) + 2b3Wv
            ts(out=fq5[:], in0=c6_q[:], scalar1=fc_sb[:, 6:7],
               scalar2=fc_sb[:, 7:8], op0=ALU.mult, op1=ALU.add)

            k_chain(1)

            # ---- scores (PE) + exp (ACT) ----------------------------------
            def fk_slices(jc):
                jh, l = divmod(jc, 4)
                lo, hi = l * 128, (l + 1) * 128
                return [
                    a_k[jh][:, 512 + lo:512 + hi],    # c1k
                    a_k[jh][:, lo:hi],                # -s1k
                    sc3_k[jh][:, 512 + lo:512 + hi],  # -c3k
                    sc3_k[jh][:, lo:hi],              # -s3k
                    c6_k[jh][:, lo:hi],               # s3k^2
                    s6_k[jh][:, lo:hi],               # s3k*c3k
                ]

            fq_list = [fq01[:, 0:256], fq01[:, 256:512],
                       fq23[:, 0:256], fq23[:, 256:512],
                       fq4[:], fq5[:]]

            ex = []
            for pr in range(4):
                sc_ps = ps_sc.tile([128, 512], f32, tag="sc")
                for half in range(2):
                    jc = pr * 2 + half
                    fks = fk_slices(jc)
                    o = sc_ps[:, half * 256:(half + 1) * 256]
                    for f in range(6):
                        nc.tensor.matmul(o, fks[f], fq_list[f],
                                         start=(f == 0), stop=(f == 5))
                e = sb.tile([128, 512], bf16, tag=f"ex{pr}")
                nc.scalar.activation(out=e[:], in_=sc_ps[:], func=AF.Exp)
                ex.append(e)

            # ---- numerator/denominator (PE) + writeback -------------------
            for ic in range(2):
                o_ps = ps_o.tile([128, VW], f32, tag=f"o{ic}")
                for jc in range(8):
                    pr, half = divmod(jc, 2)
                    lo = half * 256 + ic * 128
                    nc.tensor.matmul(o_ps[:], ex[pr][:, lo:lo + 128],
                                     vv_sb[:, jc, :],
                                     start=(jc == 0), stop=(jc == 7))
                o_sb = sb.tile([128, VW], f32, tag=f"osb{ic}")
                nc.scalar.copy(o_sb[:], o_ps[:])
                nc.sync.dma_start(out=out[ic], in_=o_sb[:])

    nc.compile()
    return nc


def _get_program():
    if "p" not in _prog_cache:
        _prog_cache["p"] = _build_program()
    return _prog_cache["p"]


def _chunkT(a2d: np.ndarray, nfree: int) -> np.ndarray:
    """(n, 256) row-major -> (128, 2, n): [p, c, n] = a2d[n, 128c + p]."""
    return np.ascontiguousarray(
        a2d.T.reshape(2, 128, nfree).transpose(1, 0, 2))


def _fit_b(F: float, sig: float, xlim: float) -> np.ndarray:
    """Density-weighted LSQ of tanh(x) ~ b1 sin(Fx)+b2 sin(3Fx)+b3 sin(6Fx)."""
    x = np.linspace(0.0, xlim, 3001)
    w = np.sqrt(np.exp(-x ** 2 / (2.0 * sig * sig)) + 2e-6)
    A = np.stack([np.sin(F * x), np.sin(3 * F * x), np.sin(6 * F * x)], 1)
    b, *_ = np.linalg.lstsq(A * w[:, None], np.tanh(x) * w, rcond=None)
    return b


def _prepare(Q_batch, K_batch, V_batch, valid_lens, Wq, Wk, Wv):
    import ml_dtypes
    BF = ml_dtypes.bfloat16

    Q = np.asarray(Q_batch, np.float32)
    K = np.asarray(K_batch, np.float32)
    V = np.asarray(V_batch, np.float32)
    L = np.asarray(valid_lens).astype(np.int64)
    Wq = np.asarray(Wq, np.float32)
    Wk = np.asarray(Wk, np.float32)
    Wv = np.asarray(Wv, np.float32)

    wqt = _chunkT(Wq, 128).astype(BF)
    wkt = _chunkT(Wk, 128).astype(BF)
    Qb = Q.astype(BF).astype(np.float32)
    Kb = K.astype(BF).astype(np.float32)
    Wqb = Wq.astype(BF).astype(np.float32)
    Wkb = Wk.astype(BF).astype(np.float32)
    HPI = float(np.pi / 2)

    in_maps = []
    for b in range(B):
        qt = _chunkT(Q[b], 256).astype(BF)
        qwk = np.concatenate([qt.reshape(128, 512), wqt.reshape(128, 256),
                              wkt.reshape(128, 256)], 1)
        kt = np.stack([_chunkT(K[b, jh * 512:(jh + 1) * 512], 512)
                       for jh in range(2)]).astype(BF)
        n = int(L[b])
        vr = np.zeros((NKV, VW), np.float32)
        vr[:n, :256] = V[b, :n]
        vr[:n, 256] = 1.0
        vvb = np.ascontiguousarray(
            vr.reshape(8, 128, VW).transpose(1, 0, 2)).astype(BF)

        # per-core adaptive base frequency: the device computes the same
        # qp/kp (bf16 products, fp32 accum); keep F*xmax + pi/2 <= pi-0.03
        qp = Qb[b] @ Wqb.T
        kp = Kb[b] @ Wkb.T
        xmax = float(max(np.abs(qp).max(), np.abs(kp).max()))
        Fb = min(F, (np.pi / 2 - 0.03) / max(xmax, 1e-6))
        sig = float(np.sqrt(qp.std() ** 2 + kp.std() ** 2))
        xlim = float(np.abs(qp).max() + np.abs(kp).max()) + 0.3
        bf_ = _fit_b(Fb, max(sig, 1e-3), xlim)
        fcoef = np.stack([
            np.full(128, Fb), np.full(128, -Fb),
            np.full(128, HPI), np.full(128, -HPI),
            bf_[0] * Wv, -bf_[1] * Wv,
            -4.0 * bf_[2] * Wv, 2.0 * bf_[2] * Wv], 1).astype(np.float32)
        in_maps.append({"qwk": qwk, "kt": kt, "vv": vvb, "fcoef": fcoef})
    return in_maps


def _gather(results) -> np.ndarray:
    outp = np.zeros((B, NQ, 256), np.float32)
    for b in range(B):
        o = results[b]["out"].astype(np.float64)  # (2, 128, VW)
        num = o[:, :, :256].reshape(NQ, 256)
        den = o[:, :, 256].reshape(NQ, 1)
        outp[b] = (num / den).astype(np.float32)
    return outp


def _install_ntff_hook():
    """Register the axon NTFF profile hook that bass_utils reads via
    antenv.axon_hooks (the shipped antenv stub lacks that module)."""
    import contextlib
    import ctypes
    import sys
    import types

    try:
        from antenv.axon_hooks import get_axon_ntff_profile_hook
        if get_axon_ntff_profile_hook() is not None:
            return
    except ImportError:
        pass

    so_path = "/opt/axon/libaxon_pjrt.so"
    if not os.path.exists(so_path):
        return
    lib = ctypes.CDLL(so_path)
    if not hasattr(lib, "axon_start_nrt_profile"):
        return
    lib.axon_start_nrt_profile.argtypes = [
        ctypes.POINTER(ctypes.c_int64), ctypes.c_size_t]
    lib.axon_start_nrt_profile.restype = ctypes.c_int64
    lib.axon_stop_nrt_profile.argtypes = [ctypes.c_char_p]
    lib.axon_stop_nrt_profile.restype = ctypes.c_int64

    @contextlib.contextmanager
    def _hook(output_dir, device_ids):
        import jax
        jax.devices()
        if device_ids:
            ids = (ctypes.c_int64 * len(device_ids))(*device_ids)
            rc = lib.axon_start_nrt_profile(ids, len(device_ids))
        else:
            rc = lib.axon_start_nrt_profile(None, 0)
        if rc != 0:
            raise RuntimeError(f"axon_start_nrt_profile rc={rc}")
        try:
            yield
        finally:
            n = lib.axon_stop_nrt_profile(str(output_dir).encode())
            print(f"ntff profile: {n} file(s) written to {output_dir}")

    mod = types.ModuleType("antenv.axon_hooks")
    mod.get_axon_ntff_profile_hook = lambda: _hook
    mod.set_axon_ntff_profile_hook = lambda h: None
    sys.modules["antenv.axon_hooks"] = mod
    import antenv
    antenv.axon_hooks = mod


def run(Q_batch, K_batch, V_batch, valid_lens, Wq, Wk, Wv,
        trace: bool = False):
    """Returns (output, exec_time_ns_or_None)."""
    from concourse.bass_utils import run_bass_kernel_spmd

    if trace:
        _install_ntff_hook()

    in_maps = _prepare(Q_batch, K_batch, V_batch, valid_lens, Wq, Wk, Wv)
    nc = _get_program()

    if os.environ.get("ADD_ATTN_SIM"):
        from concourse.bass_interp import CoreSim
        ncores = int(os.environ.get("ADD_ATTN_SIM_CORES", NCORES))
        results = []
        for c in range(ncores):
            sim = CoreSim(nc)
            for name, arr in in_maps[c].items():
                sim.tensor(name)[:] = arr
            sim.simulate()
            results.append({"out": np.array(sim.tensor("out"))})
        results += [{"out": np.ones((2, 128, VW), np.float32)}
                    ] * (NCORES - ncores)
        return _gather(results), None

    res = run_bass_kernel_spmd(nc, in_maps, core_ids=list(range(NCORES)),
                               trace=trace)
    return _gather(res.results), res.exec_time_ns


def kernel(Q_batch, K_batch, V_batch, valid_lens, Wq, Wk, Wv):
    out, _ = run(Q_batch, K_batch, V_batch, valid_lens, Wq, Wk, Wv)
    return out


# revision 22
# speedup vs baseline: 4.5411x; 1.2135x over previous
"""Additive (Bahdanau) attention on TRN2, one batch per core, SPMD over 8.

Math per batch (Q (256,256), K (1024,256), V (1024,256), H=128):
    qp = Q @ Wq.T ; kp = K @ Wk.T
    s[i,j] = sum_h Wv[h] * tanh(qp[i,h] + kp[j,h])
    out    = softmax_j(s, masked) @ V

The O(NQ*NKV*H) tanh is replaced by a 3-term sine expansion fitted to tanh
(density-weighted LSQ, wrms 7.7e-3 over the actual argument distribution):

    tanh(x) ~ b1 sin(F x) + b2 sin(3F x) + b3 sin(6F x),  F = 0.3655

sin(w(a+b)) = sin(wa)cos(wb) + cos(wa)sin(wb) makes the scores SEPARABLE:
one PE matmul with contraction 6*H = 768 instead of 33M elementwise tanh
per core.  Base features sin/cos(F*x) come from the ACT Sin table (|arg| <=
F*4.21 + pi/2 = 3.11 < pi, the table's hard valid range); the 3F and 6F
harmonics are built algebraically on DVE with fused scalar_tensor_tensor:
    s3 = (3 - 4 s1^2) s1 ; c3 = (4 c1^2 - 3) c1 ; s6 = 2 s3 c3 ; c6 = 1 - 2 s3^2
The k-side uses c6-1 (the +1 adds a per-query row constant, which softmax
cancels); the q-side folds the +1 into its coef scaling op.

Softmax uses no max-subtraction (|s| <= sum|b_m|*sum|Wv| ~ 6, exp is safe);
masked keys are handled by zeroing their V rows and ones-column on the host,
so partial numerator/denominator sums are exact.  Division happens on host.

Measured: see test.py (HW exec ~O(10us) vs 144.5us for the elementwise
baseline); rel err ~5e-3 vs the fp32 jax reference.
"""

import os
from contextlib import ExitStack

import numpy as np

B, NQ, NKV, D, H = 8, 256, 1024, 256, 128
NCORES = 8
VW = 264                 # V cols (256) + ones col (1) + pad to 264
F = 0.3655
B3 = (1.171000692830541, 0.3125350842862747, 0.0884505512829242)

_prog_cache: dict[tuple, object] = {}


def _build_program():
    import concourse.bass as bass  # noqa: F401  (registers engines)
    import concourse.tile as tile
    from concourse import bacc, mybir

    f32 = mybir.dt.float32
    bf16 = mybir.dt.bfloat16
    AF = mybir.ActivationFunctionType
    ALU = mybir.AluOpType

    nc = bacc.Bacc("TRN2", target_bir_lowering=False, debug=False,
                   num_devices=NCORES)

    # qwk = [qt(512) | wqt(256) | wkt(256)] packed for one early DMA
    qwk = nc.dram_tensor("qwk", [128, 1024], bf16, kind="ExternalInput")
    kt = nc.dram_tensor("kt", [2, 128, 2, 512], bf16, kind="ExternalInput")
    vv = nc.dram_tensor("vv", [128, 8, VW], bf16, kind="ExternalInput")
    # fcoef cols: 0:+F 1:-F 2:+pi/2 3:-pi/2 4:b1*Wv 5:-b2*Wv 6:-4b3*Wv 7:2b3*Wv
    fcoef = nc.dram_tensor("fcoef", [128, 8], f32, kind="ExternalInput")
    out = nc.dram_tensor("out", [2, 128, VW], f32, kind="ExternalOutput")

    with tile.TileContext(nc) as tc:
        with ExitStack() as ctx:
            sb = ctx.enter_context(tc.tile_pool(name="sb", bufs=1))
            ps = ctx.enter_context(
                tc.tile_pool(name="ps", bufs=1, space="PSUM"))

            # spread DMA descriptor issue across idle engines; vv is issued
            # later (dependency-deferred) so kt/qwk get the early bandwidth
            fc_sb = sb.tile([128, 8], f32)
            nc.scalar.dma_start(out=fc_sb[:], in_=fcoef[:])
            kt_sb = [sb.tile([128, 2, 512], bf16, tag=f"kt{jh}",
                             name=f"kt_sb{jh}")
                     for jh in range(2)]
            nc.sync.dma_start(out=kt_sb[0][:], in_=kt[0])
            qwk_sb = sb.tile([128, 1024], bf16)
            nc.gpsimd.dma_start(out=qwk_sb[:], in_=qwk[:])
            nc.sync.dma_start(out=kt_sb[1][:], in_=kt[1])

            # sin table preload (hides under the DMAs; fc_sb col2 = pi/2)
            dummy = sb.tile([128, 1], f32)
            nc.scalar.activation(out=dummy[:], in_=fc_sb[:, 2:3], func=AF.Sin)

            def qt_c(c):
                return qwk_sb[:, c * 256:(c + 1) * 256]

            def wqt_c(c):
                return qwk_sb[:, 512 + c * 128:512 + (c + 1) * 128]

            def wkt_c(c):
                return qwk_sb[:, 768 + c * 128:768 + (c + 1) * 128]

            # ---- projections (PE): qp/kp with h on partitions -------------
            kp_ps = []
            for jh in range(2):
                kp = ps.tile([128, 512], f32, tag=f"kp{jh}", name=f"kp{jh}")
                for c in range(2):
                    nc.tensor.matmul(kp[:], wkt_c(c), kt_sb[jh][:, c, :],
                                     start=(c == 0), stop=(c == 1))
                kp_ps.append(kp)
                if jh == 0:
                    qp_ps = ps.tile([128, 256], f32, tag="qp")
                    for c in range(2):
                        nc.tensor.matmul(qp_ps[:], wqt_c(c), qt_c(c),
                                         start=(c == 0), stop=(c == 1))

            # ---- base features (ACT Sin, args within [-pi, pi]) -----------
            # weights are pre-scaled on host so scale is the immediate F;
            # A_k[jh] = [-s1k | c1k]   (512+512)
            a_k = []
            for jh in range(2):
                ak = sb.tile([128, 1024], bf16, tag=f"ak{jh}",
                             name=f"ak{jh}")
                nc.scalar.activation(out=ak[:, 0:512], in_=kp_ps[jh][:],
                                     func=AF.Sin, scale=-F)
                nc.scalar.activation(out=ak[:, 512:1024], in_=kp_ps[jh][:],
                                     func=AF.Sin, scale=F,
                                     bias=fc_sb[:, 2:3])
                a_k.append(ak)
                if jh == 0:
                    # A_q = [s1q | -c1q]   (256+256)
                    a_q = sb.tile([128, 512], bf16)
                    nc.scalar.activation(out=a_q[:, 0:256], in_=qp_ps[:],
                                         func=AF.Sin, scale=F)
                    nc.scalar.activation(out=a_q[:, 256:512], in_=qp_ps[:],
                                         func=AF.Sin, scale=-F,
                                         bias=fc_sb[:, 3:4])

            # vv load deferred behind a_k[0] so kt/qwk own the early DMA
            # bandwidth; vv is only needed by the V-matmul much later
            vv_sb = sb.tile([128, 8, VW], bf16)
            gdum = sb.tile([128, 1], bf16)
            nc.gpsimd.tensor_copy(gdum[:], a_k[0][:, 0:1])
            nc.gpsimd.dma_start(out=vv_sb[:], in_=vv[:])

            # ---- harmonic chains (DVE: TT gets bf16 2x, TS gets 4x) -------
            tt = nc.vector.tensor_tensor
            ts = nc.vector.tensor_scalar

            sc3_k, s6_k, c6_k = [{} for _ in range(3)]

            def k_chain_head(jh):
                ak = a_k[jh]
                t_k = sb.tile([128, 1024], bf16, tag=f"tk{jh}",
                              name=f"tk{jh}")
                tt(out=t_k[:], in0=ak[:], in1=ak[:], op=ALU.mult)
                u_k = sb.tile([128, 1024], bf16, tag=f"uk{jh}",
                              name=f"uk{jh}")
                ts(out=u_k[:], in0=t_k[:], scalar1=-4.0, scalar2=3.0,
                   op0=ALU.mult, op1=ALU.add)
                sc3 = sb.tile([128, 1024], bf16, tag=f"sc3k{jh}",
                              name=f"sc3k{jh}")
                tt(out=sc3[:], in0=u_k[:], in1=ak[:], op=ALU.mult)
                sc3_k[jh] = sc3          # [-s3k | -c3k]

            def k_chain_tail(jh):
                sc3 = sc3_k[jh]
                s6 = sb.tile([128, 512], bf16, tag=f"s6k{jh}",
                             name=f"s6k{jh}")
                tt(out=s6[:], in0=sc3[:, 0:512], in1=sc3[:, 512:1024],
                   op=ALU.mult)          # = s3k*c3k
                c6 = sb.tile([128, 512], bf16, tag=f"c6k{jh}",
                             name=f"c6k{jh}")
                tt(out=c6[:], in0=sc3[:, 0:512], in1=sc3[:, 0:512],
                   op=ALU.mult)          # = s3k^2
                s6_k[jh], c6_k[jh] = s6, c6

            # DVE order tuned so the first-needed q features exist earliest
            k_chain_head(0)

            fq01 = sb.tile([128, 512], bf16)   # [s1q | -c1q] * b1Wv
            ts(out=fq01[:], in0=a_q[:], scalar1=fc_sb[:, 4:5], scalar2=None,
               op0=ALU.mult)

            t_q = sb.tile([128, 512], bf16)
            tt(out=t_q[:], in0=a_q[:], in1=a_q[:], op=ALU.mult)
            u_q = sb.tile([128, 512], bf16)
            ts(out=u_q[:], in0=t_q[:], scalar1=-4.0, scalar2=3.0,
               op0=ALU.mult, op1=ALU.add)
            sc3_q = sb.tile([128, 512], bf16)      # [s3q | c3q]
            tt(out=sc3_q[:], in0=u_q[:], in1=a_q[:], op=ALU.mult)

            fq23 = sb.tile([128, 512], bf16)   # [s3q | c3q] * (-b2Wv)
            ts(out=fq23[:], in0=sc3_q[:], scalar1=fc_sb[:, 5:6],
               scalar2=None, op0=ALU.mult)

            k_chain_tail(0)

            s6_q = sb.tile([128, 256], bf16)       # s3q*c3q
            tt(out=s6_q[:], in0=sc3_q[:, 0:256], in1=sc3_q[:, 256:512],
               op=ALU.mult)
            c6_q = sb.tile([128, 256], bf16)       # s3q^2
            tt(out=c6_q[:], in0=sc3_q[:, 0:256], in1=sc3_q[:, 0:256],
               op=ALU.mult)
            fq4 = sb.tile([128, 256], bf16)    # s3q c3q * (-4 b3Wv)
            ts(out=fq4[:], in0=s6_q[:], scalar1=fc_sb[:, 6:7], scalar2=None,
               op0=ALU.mult)
            fq5 = sb.tile([128, 256], bf16)    # s3q^2*(-4b3Wv) + 2b3Wv
            ts(out=fq5[:], in0=c6_q[:], scalar1=fc_sb[:, 6:7],
               scalar2=fc_sb[:, 7:8], op0=ALU.mult, op1=ALU.add)

            k_chain_head(1)
            k_chain_tail(1)

            # ---- scores (PE) + exp (ACT) ----------------------------------
            def fk_slices(jc):
                jh, l = divmod(jc, 4)
                lo, hi = l * 128, (l + 1) * 128
                return [
                    a_k[jh][:, 512 + lo:512 + hi],    # c1k
                    a_k[jh][:, lo:hi],                # -s1k
                    sc3_k[jh][:, 512 + lo:512 + hi],  # -c3k
                    sc3_k[jh][:, lo:hi],              # -s3k
                    c6_k[jh][:, lo:hi],               # s3k^2
                    s6_k[jh][:, lo:hi],               # s3k*c3k
                ]

            fq_list = [fq01[:, 0:256], fq01[:, 256:512],
                       fq23[:, 0:256], fq23[:, 256:512],
                       fq4[:], fq5[:]]

            ex = []
            for pr in range(4):
                sc_ps = ps.tile([128, 512], f32, tag="sc", bufs=3,
                                name="sc_ps")
                for half in range(2):
                    jc = pr * 2 + half
                    fks = fk_slices(jc)
                    o = sc_ps[:, half * 256:(half + 1) * 256]
                    for f in range(6):
                        nc.tensor.matmul(o, fks[f], fq_list[f],
                                         start=(f == 0), stop=(f == 5))
                e = sb.tile([128, 512], bf16, tag=f"ex{pr}")
                nc.scalar.activation(out=e[:], in_=sc_ps[:], func=AF.Exp)
                ex.append(e)

            # ---- numerator/denominator (PE) + writeback -------------------
            for ic in range(2):
                o_ps = ps.tile([128, VW], f32, tag=f"o{ic}", name=f"o{ic}")
                for jc in range(8):
                    pr, half = divmod(jc, 2)
                    lo = half * 256 + ic * 128
                    nc.tensor.matmul(o_ps[:], ex[pr][:, lo:lo + 128],
                                     vv_sb[:, jc, :],
                                     start=(jc == 0), stop=(jc == 7))
                o_sb = sb.tile([128, VW], f32, tag=f"osb{ic}")
                nc.vector.tensor_copy(o_sb[:], o_ps[:])
                nc.sync.dma_start(out=out[ic], in_=o_sb[:])

    nc.compile()
    return nc


def _get_program():
    if "p" not in _prog_cache:
        _prog_cache["p"] = _build_program()
    return _prog_cache["p"]


def _chunkT(a2d: np.ndarray, nfree: int) -> np.ndarray:
    """(n, 256) row-major -> (128, 2, n): [p, c, n] = a2d[n, 128c + p]."""
    return np.ascontiguousarray(
        a2d.T.reshape(2, 128, nfree).transpose(1, 0, 2))


def _fit_b(F: float, sig: float, xlim: float) -> np.ndarray:
    """Density-weighted LSQ of tanh(x) ~ b1 sin(Fx)+b2 sin(3Fx)+b3 sin(6Fx)."""
    x = np.linspace(0.0, xlim, 3001)
    w = np.sqrt(np.exp(-x ** 2 / (2.0 * sig * sig)) + 2e-6)
    A = np.stack([np.sin(F * x), np.sin(3 * F * x), np.sin(6 * F * x)], 1)
    b, *_ = np.linalg.lstsq(A * w[:, None], np.tanh(x) * w, rcond=None)
    return b


def _prepare(Q_batch, K_batch, V_batch, valid_lens, Wq, Wk, Wv):
    import ml_dtypes
    BF = ml_dtypes.bfloat16

    Q = np.asarray(Q_batch, np.float32)
    K = np.asarray(K_batch, np.float32)
    V = np.asarray(V_batch, np.float32)
    L = np.asarray(valid_lens).astype(np.int64)
    Wq = np.asarray(Wq, np.float32)
    Wk = np.asarray(Wk, np.float32)
    Wv = np.asarray(Wv, np.float32)

    wqt = _chunkT(Wq, 128).astype(BF)
    wkt = _chunkT(Wk, 128).astype(BF)
    Qb = Q.astype(BF).astype(np.float32)
    Kb = K.astype(BF).astype(np.float32)
    Wqb = Wq.astype(BF).astype(np.float32)
    Wkb = Wk.astype(BF).astype(np.float32)
    HPI = float(np.pi / 2)

    in_maps = []
    for b in range(B):
        qt = _chunkT(Q[b], 256).astype(BF)
        qwk = np.concatenate([qt.reshape(128, 512), wqt.reshape(128, 256),
                              wkt.reshape(128, 256)], 1)
        kt = np.stack([_chunkT(K[b, jh * 512:(jh + 1) * 512], 512)
                       for jh in range(2)]).astype(BF)
        n = int(L[b])
        vr = np.zeros((NKV, VW), np.float32)
        vr[:n, :256] = V[b, :n]
        vr[:n, 256] = 1.0
        vvb = np.ascontiguousarray(
            vr.reshape(8, 128, VW).transpose(1, 0, 2)).astype(BF)

        # per-core adaptive base frequency, applied by pre-scaling the
        # projection weights on host so the device sin scale stays the
        # compile-time immediate F; keep F_b*xmax + pi/2 <= pi - 0.03
        qp = Qb[b] @ Wqb.T
        kp = Kb[b] @ Wkb.T
        xmax = float(max(np.abs(qp).max(), np.abs(kp).max()))
        Fb = min(F, (np.pi / 2 - 0.03) / max(xmax, 1e-6))
        ratio = Fb / F
        if ratio < 1.0:
            qwk = qwk.copy()
            qwk[:, 512:] = (qwk[:, 512:].astype(np.float32)
                            * np.float32(ratio)).astype(BF)
        sig = float(np.sqrt(qp.std() ** 2 + kp.std() ** 2))
        xlim = float(np.abs(qp).max() + np.abs(kp).max()) + 0.3
        bf_ = _fit_b(Fb, max(sig, 1e-3), xlim)
        fcoef = np.stack([
            np.full(128, Fb), np.full(128, -Fb),
            np.full(128, HPI), np.full(128, -HPI),
            bf_[0] * Wv, -bf_[1] * Wv,
            -4.0 * bf_[2] * Wv, 2.0 * bf_[2] * Wv], 1).astype(np.float32)
        in_maps.append({"qwk": qwk, "kt": kt, "vv": vvb, "fcoef": fcoef})
    return in_maps


def _gather(results) -> np.ndarray:
    outp = np.zeros((B, NQ, 256), np.float32)
    for b in range(B):
        o = results[b]["out"].astype(np.float64)  # (2, 128, VW)
        num = o[:, :, :256].reshape(NQ, 256)
        den = o[:, :, 256].reshape(NQ, 1)
        outp[b] = (num / den).astype(np.float32)
    return outp


def _install_ntff_hook():
    """Register the axon NTFF profile hook that bass_utils reads via
    antenv.axon_hooks (the shipped antenv stub lacks that module)."""
    import contextlib
    import ctypes
    import sys
    import types

    try:
        from antenv.axon_hooks import get_axon_ntff_profile_hook
        if get_axon_ntff_profile_hook() is not None:
            return
    except ImportError:
        pass

    so_path = "/opt/axon/libaxon_pjrt.so"
    if not os.path.exists(so_path):
        return
    lib = ctypes.CDLL(so_path)
    if not hasattr(lib, "axon_start_nrt_profile"):
        return
    lib.axon_start_nrt_profile.argtypes = [
        ctypes.POINTER(ctypes.c_int64), ctypes.c_size_t]
    lib.axon_start_nrt_profile.restype = ctypes.c_int64
    lib.axon_stop_nrt_profile.argtypes = [ctypes.c_char_p]
    lib.axon_stop_nrt_profile.restype = ctypes.c_int64

    @contextlib.contextmanager
    def _hook(output_dir, device_ids):
        import jax
        jax.devices()
        if device_ids:
            ids = (ctypes.c_int64 * len(device_ids))(*device_ids)
            rc = lib.axon_start_nrt_profile(ids, len(device_ids))
        else:
            rc = lib.axon_start_nrt_profile(None, 0)
        if rc != 0:
            raise RuntimeError(f"axon_start_nrt_profile rc={rc}")
        try:
            yield
        finally:
            n = lib.axon_stop_nrt_profile(str(output_dir).encode())
            print(f"ntff profile: {n} file(s) written to {output_dir}")

    mod = types.ModuleType("antenv.axon_hooks")
    mod.get_axon_ntff_profile_hook = lambda: _hook
    mod.set_axon_ntff_profile_hook = lambda h: None
    sys.modules["antenv.axon_hooks"] = mod
    import antenv
    antenv.axon_hooks = mod


def run(Q_batch, K_batch, V_batch, valid_lens, Wq, Wk, Wv,
        trace: bool = False):
    """Returns (output, exec_time_ns_or_None)."""
    from concourse.bass_utils import run_bass_kernel_spmd

    if trace:
        _install_ntff_hook()

    in_maps = _prepare(Q_batch, K_batch, V_batch, valid_lens, Wq, Wk, Wv)
    nc = _get_program()

    if os.environ.get("ADD_ATTN_SIM"):
        from concourse.bass_interp import CoreSim
        ncores = int(os.environ.get("ADD_ATTN_SIM_CORES", NCORES))
        results = []
        for c in range(ncores):
            sim = CoreSim(nc)
            for name, arr in in_maps[c].items():
                sim.tensor(name)[:] = arr
            sim.simulate()
            results.append({"out": np.array(sim.tensor("out"))})
        results += [{"out": np.ones((2, 128, VW), np.float32)}
                    ] * (NCORES - ncores)
        return _gather(results), None

    res = run_bass_kernel_spmd(nc, in_maps, core_ids=list(range(NCORES)),
                               trace=trace)
    return _gather(res.results), res.exec_time_ns


def kernel(Q_batch, K_batch, V_batch, valid_lens, Wq, Wk, Wv):
    out, _ = run(Q_batch, K_batch, V_batch, valid_lens, Wq, Wk, Wv)
    return out
